# revision 5
# baseline (speedup 1.0000x reference)
"""DGCNN (SGConv K=2 + conv-pool + fc) Trainium2 kernel.

Math:
  A_norm = D^-1/2 (A + I) D^-1/2   (A from tril edge_w, symmetrized)
  h      = relu(A_norm^2 @ x @ lin_w + lin_b)        [B, N, H]
  pooled = relu(einsum('bnh,n->bh', h, conv_w) + conv_b)
  out    = pooled @ fc_w + fc_b                      [B, C]

Device strategy (data-parallel over batch, 8 cores x 512 batches):
  Host folds the two SGConv hops into A2 = A_norm @ A_norm and folds
  |conv_w| into A2's columns (c*relu(z) = sign(c)*relu(|c|*z)).

  Per 16-batch iteration (all 16-bit tensors fp16; PSUM fp32):
    MM_L x16: z[j, bh]  = x_b @ lin_w      (lhsT = xT_b slice)  -> 2 PSUM banks
    copy    : z  PSUM -> SBUF fp16         (split DVE/ACT)
    MM_A x8 : u'[bh, i] = z_pair^T @ A2c   (lhsT = z 2-batch block,
              moving = A2c with columns permuted sign-pos-first)
    ACT     : u = relu(u') PSUM -> SBUF fp16
    DVE     : tensor_reduce over i: full-sum and pos-sum -> PP columns
  Node pooling is a free-dim reduction (pooled = 2*pos - full), so the
  old [128,1]-stationary pooling matmul, its PSUM copies, and the
  per-4-group 8KB output DMAs are all gone; one 256KB DMA ships PP at
  the end. x is DMA'd in 2 MB chunks on the sync (HWDGE) queue.

  Host epilogue: relu(pooled + conv_b) @ fc_w + fc_b  on [B, 64].
"""

import ml_dtypes
import numpy as np

import concourse.bacc as bacc
import concourse.bass as bass
import concourse.mybir as mybir
import concourse.tile as tile
from concourse.bass_utils import run_bass_kernel_spmd

N = 128       # nodes
F_IN = 128    # in features
H = 64        # hidden
C = 40        # classes
B = 4096      # batch
NCORES = 8
BPC = B // NCORES          # 512 batches per core
G = 16                     # batches per iteration (2 PSUM banks: 16*64 fp32)
NPAIR = G // 2             # 2-batch pairs per iteration
NG = BPC // G              # 32 iterations
CHUNK = 4                  # iterations per x DMA (4*16 batches = 2 MB)
NCHUNK = NG // CHUNK
NQ = BPC // 2              # total 2-batch pairs per core (PP columns)

F32 = mybir.dt.float32
BF16 = mybir.dt.bfloat16
FP16 = mybir.dt.float16
RELU = mybir.ActivationFunctionType.Relu
COPY = mybir.ActivationFunctionType.Copy
AXIS_X = mybir.AxisListType.X
ADD = mybir.AluOpType.add

MM_DT = FP16
X_DT = FP16
ZACT = 256   # zst columns copied by ACT (rest by DVE) per iteration

_PROG_CACHE: dict = {}
_last_in_maps: list = []
_NPOS = 57   # baked reduce split; set from conv_w before building

# ablation: 'full', 'no_r' (skip reduces), 'no_a' (skip MM_A+relu too),
# 'lin_only' (just MM_L + copy), 'dma_only'
_VARIANT = "full"


def _build_program(has_bias: bool, repeat: int = 1):
    npos = _NPOS
    nc = bacc.Bacc(
        "TRN2", target_bir_lowering=False, debug=False, num_devices=NCORES
    )
    xP = nc.declare_dram_parameter(
        "xP", [NCHUNK, F_IN, CHUNK * G, N], X_DT, isOutput=False
    )
    a2c = nc.declare_dram_parameter("a2c", [N, N], MM_DT, isOutput=False)
    linw = nc.declare_dram_parameter("linw", [F_IN, H], X_DT, isOutput=False)
    if has_bias:
        btile = nc.declare_dram_parameter("btile", [N, G * H], F32, isOutput=False)
    # [:, 0:NQ] = full sums, [:, NQ:2*NQ] = positive-block sums
    pooledpn = nc.declare_dram_parameter("pooledpn", [N, 2 * NQ], F32, isOutput=True)

    with tile.TileContext(nc) as tc:
        with (
            tc.tile_pool(name="const", bufs=1) as constp,
            tc.tile_pool(name="xin", bufs=3) as xinp,
            tc.tile_pool(name="zs", bufs=3) as zsp,
            tc.tile_pool(name="u", bufs=3) as up,
            tc.tile_pool(name="ppp", bufs=1) as ppp,
            tc.tile_pool(name="psL", bufs=2, space="PSUM") as psL,
            tc.tile_pool(name="psA", bufs=2, space="PSUM") as psA,
        ):
            a2c_t = constp.tile([N, N], MM_DT)
            nc.sync.dma_start(a2c_t[:], a2c[:, :])
            linw_t = constp.tile([F_IN, H], X_DT)
            nc.sync.dma_start(linw_t[:], linw[:, :])
            if has_bias:
                bt_t = constp.tile([N, G * H], F32)
                nc.sync.dma_start(bt_t[:], btile[:, :])

            import contextlib

            loop_cm = (
                tc.For_i(0, repeat, 1) if repeat > 1 else contextlib.nullcontext()
            )

            with loop_cm:
                PP = ppp.tile([N, 2 * NQ], F32, name="PP", tag="PP")

                zst_q: dict = {}
                ups_q: dict = {}
                u_q: dict = {}
                X_cur: list = [None]

                def stage_L(i):
                    if i % CHUNK == 0:
                        X8 = xinp.tile(
                            [F_IN, CHUNK * G * N], X_DT, name="X8", tag="X"
                        )
                        nc.sync.dma_start(
                            X8[:].rearrange("p (b j) -> p b j", b=CHUNK * G),
                            xP[i // CHUNK],
                        )
                        X_cur[0] = X8
                    X = X_cur[0]
                    off = (i % CHUNK) * G * N
                    zps = psL.tile([N, G * H], F32, tag="zps")
                    for b in range(G):
                        nc.tensor.matmul(
                            zps[:, b * H : (b + 1) * H],
                            lhsT=X[:, off + b * N : off + (b + 1) * N],
                            rhs=linw_t[:],
                            start=True,
                            stop=True,
                        )
                    zst = zsp.tile([N, G * H], MM_DT, tag="zst")
                    split = G * H - ZACT
                    if split > 0:
                        nc.vector.tensor_copy(zst[:, 0:split], zps[:, 0:split])
                    if ZACT > 0:
                        nc.scalar.copy(zst[:, split:], zps[:, split:])
                    zst_q[i] = zst

                def stage_A(i):
                    zst = zst_q.pop(i)
                    ups = psA.tile([N, NPAIR * N], F32, tag="ups")
                    for p in range(NPAIR):
                        nc.tensor.matmul(
                            ups[:, p * N : (p + 1) * N],
                            lhsT=zst[:, p * N : (p + 1) * N],
                            rhs=a2c_t[:],
                            start=True,
                            stop=True,
                        )
                    ups_q[i] = ups

                def stage_U(i):
                    ups = ups_q.pop(i)
                    ut = up.tile([N, NPAIR * N], MM_DT, tag="ut")
                    if has_bias:
                        zb = zsp.tile([N, NPAIR * N], F32, tag="zb")
                        nc.vector.tensor_add(zb[:], ups[:], bt_t[:])
                        nc.scalar.activation(ut[:], zb[:], RELU)
                    else:
                        nc.scalar.activation(ut[:], ups[:], RELU)
                    u_q[i] = ut

                def stage_R(i):
                    ut = u_q.pop(i)
                    u3 = ut[:].rearrange("m (q i) -> m q i", q=NPAIR)
                    nc.vector.tensor_reduce(
                        PP[:, i * NPAIR : (i + 1) * NPAIR],
                        u3,
                        axis=AXIS_X,
                        op=ADD,
                    )
                    if 0 < npos < N:
                        nc.vector.tensor_reduce(
                            PP[:, NQ + i * NPAIR : NQ + (i + 1) * NPAIR],
                            u3[:, :, 0:npos],
                            axis=AXIS_X,
                            op=ADD,
                        )

                if _VARIANT == "dma_only":
                    for c in range(NCHUNK):
                        X8 = xinp.tile(
                            [F_IN, CHUNK * G * N], X_DT, name="X8d", tag="X"
                        )
                        nc.sync.dma_start(
                            X8[:].rearrange("p (b j) -> p b j", b=CHUNK * G),
                            xP[c],
                        )
                        if c == NCHUNK - 1:
                            nc.vector.tensor_copy(
                                PP[:, 0:128], X8[:, 0:256].bitcast(F32)
                            )
                else:
                    run_A = _VARIANT in ("full", "no_r", "no_a")
                    run_U = _VARIANT in ("full", "no_r")
                    run_R = _VARIANT == "full"
                    # Per-engine queue order matters: ops whose inputs are
                    # oldest go first so a wait on fresh data never blocks
                    # ready work behind it (ACT: relu(i-1) before zstA(i);
                    # DVE: reduces(i-2) before zstD(i)).
                    for i in range(NG + 3):
                        if 2 <= i < NG + 2 and run_U:
                            stage_U(i - 2)
                        if i >= 3 and i - 3 < NG and run_R:
                            stage_R(i - 3)
                        if _VARIANT == "no_r" and 2 <= i < NG + 2:
                            u_q.pop(i - 2)
                        if i < NG:
                            stage_L(i)
                        if 1 <= i < NG + 1 and run_A:
                            if _VARIANT == "no_a":
                                zst_q.pop(i - 1)
                            else:
                                stage_A(i - 1)
                    if _VARIANT == "lin_only":
                        for k in list(zst_q):
                            zst_q.pop(k)
                    if _VARIANT in ("lin_only", "no_a", "no_r"):
                        # bind PP so the output DMA has a producer
                        nc.vector.memset(PP[:, 0 : 2 * NQ], 0.0)

                nc.sync.dma_start(pooledpn[:, :], PP[:])
    nc.compile()
    return nc


def _get_program(has_bias: bool):
    key = (has_bias, MM_DT, _NPOS, _VARIANT)
    if key not in _PROG_CACHE:
        _PROG_CACHE[key] = _build_program(has_bias)
    return _PROG_CACHE[key]


def _host_adjacency(edge_w, conv_w):
    """A2 with |c| folded into columns and columns permuted pos-sign-first."""
    ew = np.asarray(edge_w, dtype=np.float64)
    A = np.zeros((N, N), dtype=np.float64)
    xs, ys = np.tril_indices(N)
    A[xs, ys] = ew
    A = A + A.T - np.diag(np.diag(A))
    Ah = A + np.eye(N)
    deg = Ah.sum(axis=1)
    dinv = np.where(deg > 0, deg ** -0.5, 0.0)
    An = dinv[:, None] * Ah * dinv[None, :]
    A2 = An @ An
    c = np.asarray(conv_w, dtype=np.float64)
    # a2c[j, i] = A2[i, j] * |c_i| = A2[j, i] * |c_i| (A2 symmetric)
    a2c = A2 * np.abs(c)[None, :]
    s = np.sign(c)
    perm = np.concatenate([np.where(s > 0)[0], np.where(s <= 0)[0]])
    npos = int((s > 0).sum())
    a2cp = np.ascontiguousarray(a2c[:, perm]).astype(np.float32)
    return a2cp, npos


def _run(inputs: dict, trace: bool = False):
    global _NPOS
    x = np.asarray(inputs["x"], dtype=np.float32)
    edge_w = np.asarray(inputs["edge_w"], dtype=np.float32)
    lin_w = np.ascontiguousarray(np.asarray(inputs["lin_w"], dtype=np.float32))
    lin_b = np.asarray(inputs["lin_b"], dtype=np.float32)
    conv_w = np.asarray(inputs["conv_w"], dtype=np.float32)
    conv_b = np.asarray(inputs["conv_b"], dtype=np.float32)
    fc_w = np.asarray(inputs["fc_w"], dtype=np.float32)
    fc_b = np.asarray(inputs["fc_b"], dtype=np.float32)

    a2cp, npos = _host_adjacency(edge_w, conv_w)
    _NPOS = npos
    has_bias = bool(np.any(lin_b != 0))
    nc = _get_program(has_bias)

    _np_of = {F32: np.float32, BF16: ml_dtypes.bfloat16, FP16: np.float16}
    np_xdt = _np_of[X_DT]
    np_mmdt = _np_of[MM_DT]
    linw_dev = lin_w.astype(np_xdt)
    a2cp_dev = a2cp.astype(np_mmdt)
    in_maps = []
    for k in range(NCORES):
        xc = x[k * BPC : (k + 1) * BPC]                  # [512, j, f]
        xc = xc.reshape(NCHUNK, CHUNK * G, N, F_IN)      # [c, b, j, f]
        xPk = np.ascontiguousarray(
            xc.transpose(0, 3, 1, 2).astype(np_xdt)
        )  # [c, f, b, j]
        m = {"xP": xPk, "a2c": a2cp_dev, "linw": linw_dev}
        if has_bias:
            # bias term in [m=(b2,h), i] layout: |c_perm(i)| * lin_b[h]
            c = np.asarray(conv_w, dtype=np.float64)
            s = np.sign(c)
            perm = np.concatenate([np.where(s > 0)[0], np.where(s <= 0)[0]])
            cp = np.abs(c)[perm]                          # [i]
            bt = np.zeros((N, G * H), dtype=np.float64)
            for m_ in range(N):
                h = m_ % H
                bt[m_, :] = np.tile(
                    lin_b.astype(np.float64)[h] * cp, NPAIR
                )
            m["btile"] = np.ascontiguousarray(bt.astype(np.float32))
        in_maps.append(m)

    global _last_in_maps
    _last_in_maps = in_maps
    try:
        res = run_bass_kernel_spmd(nc, in_maps, list(range(NCORES)), trace=trace)
    except ModuleNotFoundError:
        res = run_bass_kernel_spmd(nc, in_maps, list(range(NCORES)), trace=False)

    # unpack: PP[m, q] m=(b2*64+h), q=global pair; pooled[2q+b2, h]
    pooled_parts = []
    for k in range(NCORES):
        pp = res.results[k]["pooledpn"]                  # [128, 2*NQ]
        full = pp[:, 0:NQ].astype(np.float64)
        pos = pp[:, NQ : 2 * NQ].astype(np.float64)
        if npos == 0:
            p2 = -full
        elif npos == N:
            p2 = full
        else:
            p2 = 2.0 * pos - full
        pooled_parts.append(
            p2.reshape(2, H, NQ).transpose(2, 0, 1).reshape(BPC, H)
        )
    pooled = np.concatenate(pooled_parts, axis=0).astype(np.float32)  # [B, H]

    p = np.maximum(pooled + conv_b[0], 0.0).astype(np.float32)
    out = (p @ fc_w + fc_b).astype(np.float32)
    return out, res


def kernel(x, edge_w, lin_w, lin_b, conv_w, conv_b, fc_w, fc_b):
    out, _ = _run(
        {
            "x": x,
            "edge_w": edge_w,
            "lin_w": lin_w,
            "lin_b": lin_b,
            "conv_w": conv_w,
            "conv_b": conv_b,
            "fc_w": fc_w,
            "fc_b": fc_b,
        }
    )
    return out


# revision 9
# speedup vs baseline: 1.6034x; 1.6034x over previous
"""DGCNN (SGConv K=2 + conv-pool + fc) Trainium2 kernel.

Math:
  A_norm = D^-1/2 (A + I) D^-1/2   (A from tril edge_w, symmetrized)
  h      = relu(A_norm^2 @ x @ lin_w + lin_b)        [B, N, H]
  pooled = relu(einsum('bnh,n->bh', h, conv_w) + conv_b)
  out    = pooled @ fc_w + fc_b                      [B, C]

Device strategy (data-parallel over batch, 8 cores x 512 batches):
  Host folds the two SGConv hops into A2 = A_norm @ A_norm and folds the
  SIGNED conv weight c into A2's columns, so the A2-hop matmul outputs
  w[i] = c_i * z2[i].  With s = sign(c):
      pooled = sum_i s_i relu(|c_i| z2_i) = 1/2 (sum_i w_i + sum_i s_i |w_i|)
  The first term is a LINEAR functional of x -> host computes it directly
  as (x . q) @ lin_w with q = A2 @ c (one cheap numpy pass).  The second
  term is two abs-valued free-dim reduces (A2 columns are permuted
  pos-signs-first) read straight from PSUM -- no relu, no pooling matmul.

  Per 16-batch iteration (fp16 operands; PSUM fp32):
    PE : MM_L x16: z[j, bh]  = x_b @ lin_w     (lhsT = xT_b slice)
         MM_A x8 : w[bh, i]  = z_pair^T @ A2c  (lhsT = z 2-batch block)
    ACT: z PSUM -> SBUF fp16 (feeds MM_A's stationary operand)
    DVE: tensor_reduce(|.|) over i in [0,npos) and [npos,128) -> PP cols
  One 256KB DMA ships PP at the end; x streams in 2 MB chunks (sync ring).

  Host epilogue: pooled = (wsum + pos - neg)/2; relu(pooled + conv_b) @ fc_w.
"""

import ml_dtypes
import numpy as np

import concourse.bacc as bacc
import concourse.bass as bass
import concourse.mybir as mybir
import concourse.tile as tile
from concourse.bass_utils import run_bass_kernel_spmd

N = 128       # nodes
F_IN = 128    # in features
H = 64        # hidden
C = 40        # classes
B = 4096      # batch
NCORES = 8
BPC = B // NCORES          # 512 batches per core
G = 16                     # batches per iteration (2 PSUM banks: 16*64 fp32)
NPAIR = G // 2             # 2-batch pairs per iteration
NG = BPC // G              # 32 iterations
CHUNK = 4                  # iterations per x DMA (4*16 batches = 2 MB)
NCHUNK = NG // CHUNK
NQ = BPC // 2              # total 2-batch pairs per core (PP columns)

F32 = mybir.dt.float32
BF16 = mybir.dt.bfloat16
FP16 = mybir.dt.float16
RELU = mybir.ActivationFunctionType.Relu
AXIS_X = mybir.AxisListType.X
ADD = mybir.AluOpType.add

MM_DT = FP16
X_DT = FP16

_PROG_CACHE: dict = {}
_last_in_maps: list = []
_NPOS = 57   # baked reduce split; set from conv_w before building

# ablation: 'full', 'no_r' (skip reduces), 'lin_only' (MM_L + copy),
# 'dma_only'
_VARIANT = "full"


def _build_program(has_bias: bool, repeat: int = 1):
    npos = _NPOS
    nc = bacc.Bacc(
        "TRN2", target_bir_lowering=False, debug=False, num_devices=NCORES
    )
    xP = nc.declare_dram_parameter(
        "xP", [NCHUNK, F_IN, CHUNK * G, N], X_DT, isOutput=False
    )
    a2c = nc.declare_dram_parameter("a2c", [N, N], MM_DT, isOutput=False)
    linw = nc.declare_dram_parameter("linw", [F_IN, H], X_DT, isOutput=False)
    if has_bias:
        btile = nc.declare_dram_parameter("btile", [N, NPAIR * N], F32, isOutput=False)
    # [:, 0:NQ] = abs-sums over pos-sign block, [:, NQ:2NQ] = neg block
    pooledpn = nc.declare_dram_parameter("pooledpn", [N, 2 * NQ], F32, isOutput=True)

    with tile.TileContext(nc) as tc:
        with (
            tc.tile_pool(name="const", bufs=1) as constp,
            tc.tile_pool(name="xin", bufs=3) as xinp,
            tc.tile_pool(name="zs", bufs=3) as zsp,
            tc.tile_pool(name="ppp", bufs=1) as ppp,
            tc.tile_pool(name="psL", bufs=2, space="PSUM") as psL,
            tc.tile_pool(name="psA", bufs=4, space="PSUM") as psA,
        ):
            a2c_t = constp.tile([N, N], MM_DT)
            nc.sync.dma_start(a2c_t[:], a2c[:, :])
            linw_t = constp.tile([F_IN, H], X_DT)
            nc.sync.dma_start(linw_t[:], linw[:, :])
            if has_bias:
                bt_t = constp.tile([N, NPAIR * N], F32)
                nc.sync.dma_start(bt_t[:], btile[:, :])

            import contextlib

            loop_cm = (
                tc.For_i(0, repeat, 1) if repeat > 1 else contextlib.nullcontext()
            )

            with loop_cm:
                PP = ppp.tile([N, 2 * NQ], F32, name="PP", tag="PP")

                zst_q: dict = {}
                ups_q: dict = {}
                X_cur: list = [None]

                def stage_L(i):
                    if i % CHUNK == 0:
                        X8 = xinp.tile(
                            [F_IN, CHUNK * G * N], X_DT, name="X8", tag="X"
                        )
                        nc.sync.dma_start(
                            X8[:].rearrange("p (b j) -> p b j", b=CHUNK * G),
                            xP[i // CHUNK],
                        )
                        X_cur[0] = X8
                    X = X_cur[0]
                    off = (i % CHUNK) * G * N
                    zps = psL.tile([N, G * H], F32, tag="zps")
                    for b in range(G):
                        nc.tensor.matmul(
                            zps[:, b * H : (b + 1) * H],
                            lhsT=X[:, off + b * N : off + (b + 1) * N],
                            rhs=linw_t[:],
                            start=True,
                            stop=True,
                        )
                    zst = zsp.tile([N, G * H], MM_DT, tag="zst")
                    nc.scalar.copy(zst[:], zps[:])
                    zst_q[i] = zst

                HP = NPAIR // 2  # pairs per PSUM-bank half

                def stage_A(i, half):
                    zst = zst_q[i]
                    if half == 1:
                        zst_q.pop(i)
                    ups = psA.tile([N, HP * N], F32, tag="ups")
                    for p in range(HP):
                        pp_ = half * HP + p
                        nc.tensor.matmul(
                            ups[:, p * N : (p + 1) * N],
                            lhsT=zst[:, pp_ * N : (pp_ + 1) * N],
                            rhs=a2c_t[:],
                            start=True,
                            stop=True,
                        )
                    ups_q[(i, half)] = ups

                def stage_R(i, half):
                    ups = ups_q.pop((i, half))
                    if has_bias:
                        wb = zsp.tile([N, HP * N], F32, tag="wb")
                        nc.vector.tensor_add(
                            wb[:], ups[:], bt_t[:, half * HP * N : (half + 1) * HP * N]
                        )
                        src = wb
                    else:
                        src = ups
                    u3 = src[:].rearrange("m (q i) -> m q i", q=HP)
                    col = i * NPAIR + half * HP
                    if npos > 0:
                        nc.vector.tensor_reduce(
                            PP[:, col : col + HP],
                            u3[:, :, 0:npos],
                            axis=AXIS_X,
                            op=ADD,
                            apply_absolute_value=True,
                        )
                    if npos < N:
                        nc.vector.tensor_reduce(
                            PP[:, NQ + col : NQ + col + HP],
                            u3[:, :, npos:N],
                            axis=AXIS_X,
                            op=ADD,
                            apply_absolute_value=True,
                        )

                if _VARIANT == "dma_only":
                    for c in range(NCHUNK):
                        X8 = xinp.tile(
                            [F_IN, CHUNK * G * N], X_DT, name="X8d", tag="X"
                        )
                        nc.sync.dma_start(
                            X8[:].rearrange("p (b j) -> p b j", b=CHUNK * G),
                            xP[c],
                        )
                        if c == NCHUNK - 1:
                            nc.vector.tensor_copy(
                                PP[:, 0:128], X8[:, 0:256].bitcast(F32)
                            )
                else:
                    run_A = _VARIANT in ("full", "no_r")
                    run_R = _VARIANT == "full"
                    # Engine queue order: reduces (oldest deps) before this
                    # iteration's matmuls/copies so a wait on fresh data never
                    # blocks ready work behind it.
                    for i in range(NG + 2):
                        if i >= 2 and i - 2 < NG and run_R:
                            stage_R(i - 2, 0)
                            stage_R(i - 2, 1)
                        if _VARIANT == "no_r" and 2 <= i < NG + 2:
                            ups_q.pop((i - 2, 0))
                            ups_q.pop((i - 2, 1))
                        if i < NG:
                            stage_L(i)
                        if 1 <= i < NG + 1 and run_A:
                            stage_A(i - 1, 0)
                            stage_A(i - 1, 1)
                    if _VARIANT == "lin_only":
                        for k in list(zst_q):
                            zst_q.pop(k)
                    if _VARIANT in ("lin_only", "no_r"):
                        nc.vector.memset(PP[:, 0 : 2 * NQ], 0.0)

                nc.sync.dma_start(pooledpn[:, :], PP[:])
    nc.compile()
    return nc


def _get_program(has_bias: bool):
    key = (has_bias, MM_DT, _NPOS, _VARIANT)
    if key not in _PROG_CACHE:
        _PROG_CACHE[key] = _build_program(has_bias)
    return _PROG_CACHE[key]


def _host_adjacency(edge_w, conv_w):
    """A2 with signed c folded into columns, permuted pos-sign-first; and
    q = A2 @ c for the host-side linear term."""
    ew = np.asarray(edge_w, dtype=np.float64)
    A = np.zeros((N, N), dtype=np.float64)
    xs, ys = np.tril_indices(N)
    A[xs, ys] = ew
    A = A + A.T - np.diag(np.diag(A))
    Ah = A + np.eye(N)
    deg = Ah.sum(axis=1)
    dinv = np.where(deg > 0, deg ** -0.5, 0.0)
    An = dinv[:, None] * Ah * dinv[None, :]
    A2 = An @ An
    c = np.asarray(conv_w, dtype=np.float64)
    a2cs = A2 * c[None, :]              # a2cs[j, i] = A2[j, i] * c_i
    q = A2 @ c                          # q_j = sum_i A2[j, i] c_i
    s = np.sign(c)
    perm = np.concatenate([np.where(s > 0)[0], np.where(s <= 0)[0]])
    npos = int((s > 0).sum())
    a2cp = np.ascontiguousarray(a2cs[:, perm]).astype(np.float32)
    return a2cp, q, perm, npos


def _run(inputs: dict, trace: bool = False):
    global _NPOS
    x = np.asarray(inputs["x"], dtype=np.float32)
    edge_w = np.asarray(inputs["edge_w"], dtype=np.float32)
    lin_w = np.ascontiguousarray(np.asarray(inputs["lin_w"], dtype=np.float32))
    lin_b = np.asarray(inputs["lin_b"], dtype=np.float32)
    conv_w = np.asarray(inputs["conv_w"], dtype=np.float32)
    conv_b = np.asarray(inputs["conv_b"], dtype=np.float32)
    fc_w = np.asarray(inputs["fc_w"], dtype=np.float32)
    fc_b = np.asarray(inputs["fc_b"], dtype=np.float32)

    a2cp, q, perm, npos = _host_adjacency(edge_w, conv_w)
    _NPOS = npos
    has_bias = bool(np.any(lin_b != 0))
    nc = _get_program(has_bias)

    _np_of = {F32: np.float32, BF16: ml_dtypes.bfloat16, FP16: np.float16}
    np_xdt = _np_of[X_DT]
    np_mmdt = _np_of[MM_DT]
    linw_dev = lin_w.astype(np_xdt)
    a2cp_dev = a2cp.astype(np_mmdt)
    in_maps = []
    for k in range(NCORES):
        xc = x[k * BPC : (k + 1) * BPC]                  # [512, j, f]
        xc = xc.reshape(NCHUNK, CHUNK * G, N, F_IN)      # [c, b, j, f]
        xPk = np.ascontiguousarray(
            xc.transpose(0, 3, 1, 2).astype(np_xdt)
        )  # [c, f, b, j]
        m = {"xP": xPk, "a2c": a2cp_dev, "linw": linw_dev}
        if has_bias:
            # bias term in [m=(b2,h), (p,i)] layout: c_perm(i) * lin_b[h]
            cp = np.asarray(conv_w, dtype=np.float64)[perm]
            col = np.tile(cp, NPAIR)                      # [(p, i)]
            row = np.tile(lin_b.astype(np.float64), 2)    # [m]
            m["btile"] = np.ascontiguousarray(
                np.outer(row, col).astype(np.float32)
            )
        in_maps.append(m)

    global _last_in_maps
    _last_in_maps = in_maps
    try:
        res = run_bass_kernel_spmd(nc, in_maps, list(range(NCORES)), trace=trace)
    except ModuleNotFoundError:
        res = run_bass_kernel_spmd(nc, in_maps, list(range(NCORES)), trace=False)

    # Host linear term: wsum[b, h] = sum_j q_j z[b, j, h] = ((x . q) @ lin_w)
    y = np.tensordot(x, q.astype(np.float32), axes=([1], [0]))   # [B, F]
    wsum = y @ lin_w                                             # [B, H]
    if has_bias:
        wsum = wsum + float(np.sum(conv_w.astype(np.float64))) * lin_b[None, :]

    # unpack: PP[m, q] m=(b2*64+h), q=global pair; pooled[2q+b2, h]
    pooled_parts = []
    for k in range(NCORES):
        pp = res.results[k]["pooledpn"].astype(np.float64)       # [128, 2NQ]
        pos = pp[:, 0:NQ] if npos > 0 else 0.0
        neg = pp[:, NQ : 2 * NQ] if npos < N else 0.0
        sabs = pos - neg                                          # [128, NQ]
        pooled_parts.append(
            np.asarray(sabs).reshape(2, H, NQ).transpose(2, 0, 1).reshape(BPC, H)
        )
    sabs_all = np.concatenate(pooled_parts, axis=0)               # [B, H]
    pooled = 0.5 * (wsum.astype(np.float64) + sabs_all)

    p = np.maximum(pooled + conv_b[0], 0.0).astype(np.float32)
    out = (p @ fc_w + fc_b).astype(np.float32)
    return out, res


def kernel(x, edge_w, lin_w, lin_b, conv_w, conv_b, fc_w, fc_b):
    out, _ = _run(
        {
            "x": x,
            "edge_w": edge_w,
            "lin_w": lin_w,
            "lin_b": lin_b,
            "conv_w": conv_w,
            "conv_b": conv_b,
            "fc_w": fc_w,
            "fc_b": fc_b,
        }
    )
    return out
